# revision 1
# baseline (speedup 1.0000x reference)
"""Trainium2 Bass kernel for the BallActor GNN (EdgeConv over fully-connected
per-sample graphs, batch 1024 x 21 objects).

Key algorithmic facts exploited:
  * knn_actor K=20 over NOBJ=21 with self masked => the "kNN graph" is simply
    ALL ordered pairs (i, j != i); top_k is unnecessary and max-aggregation is
    order independent.
  * EdgeConv first layer is linear in [x_i, x_j - x_i]:
        h(i,j) = x_i @ (A - B) + x_j @ B + bm1   (Wm1 = [[A],[B]])
    so per-node terms u_i = x_i@(A-B), v_j = x_j@B are precomputed and each
    edge costs only an elementwise add + tanh + the second 128x128 matmul.
  * The class embedding path collapses to a 3-row table lookup, folded into
    u/v via one-hot rows (host precomputes F3 = tanh(tanh(emb)@We + be) and
    G = F3 @ W_cls).
  * Edges are enumerated as 20 cyclic shifts d=1..20: j = (i+d) mod 21.  With
    v stored duplicated along the object axis ([S, 41]), every shift is a
    single strided access pattern and msg columns align exactly with agg
    columns => running elementwise max, no gather/top_k at all.

Sharding: pure data parallel over the batch: 1024 samples -> 8 cores x 128.
Params are replicated; outputs are concatenated on host.
"""

import os
import numpy as np
import ml_dtypes

BS = 1024
NOBJ = 21
HID = 128
EMB = 64
NCORES = 8
S = BS // NCORES          # samples per core
N = S * NOBJ              # nodes per core (2688)
CH = 24                   # samples per matmul chunk (24*21 = 504 <= 512)
F32 = np.float32
BF16 = ml_dtypes.bfloat16

# weight-pack column layout (single [128, WCOLS] tensor, one DMA)
_OFF_WS2 = 0
_OFF_WUS = 128
_OFF_WVS = 256
_OFF_WM2 = 384
_OFF_WA1 = 512
_OFF_WA2 = 640            # 4 cols
_OFF_WS1 = 644            # 4 rows of 128, on partition 0
_OFF_GU = 644 + 4 * 128   # 3 rows of 128, on partition 0
_OFF_GV = _OFF_GU + 3 * 128
WCOLS = _OFF_GV + 3 * 128

_cache = {}


def _build_nc(edge_dt_name: str):
    import concourse.bass as bass  # noqa: F401
    import concourse.bacc as bacc
    import concourse.tile as tile
    from concourse import mybir

    dt = mybir.dt
    edt = getattr(dt, edge_dt_name)
    AF = mybir.ActivationFunctionType
    OP = mybir.AluOpType

    nc = bacc.Bacc("TRN2")

    # ---------------- DRAM I/O ----------------
    d_state = nc.dram_tensor("state", [S, 63], dt.float32, kind="ExternalInput")
    d_tar = nc.dram_tensor("tar", [S, NOBJ * 2], dt.float32, kind="ExternalInput")
    d_wpack = nc.dram_tensor("wpack", [HID, WCOLS], edt, kind="ExternalInput")
    # bias rows: bs1, bs2, bm1, bm2, ba1, ba2_mu(pad), ba2_ls(pad)
    d_bias = nc.dram_tensor("biases", [7, HID], dt.float32, kind="ExternalInput")
    d_out = nc.dram_tensor("out", [S, 4 * NOBJ], dt.float32, kind="ExternalOutput")

    nchunk = (S + CH - 1) // CH  # 6 chunks: 5 full + one 8-sample remainder

    def chunks():
        for k in range(nchunk):
            s0 = k * CH
            ns = min(CH, S - s0)
            yield s0, ns

    with tile.TileContext(nc) as tc, \
         tc.tile_pool(name="per", bufs=1) as per, \
         tc.tile_pool(name="edge", bufs=4) as edge:

        # ---- persistent tiles ----
        wpack = per.tile([HID, WCOLS], edt, tag="wpack")
        nc.sync.dma_start(out=wpack, in_=d_wpack[:])
        w_Ws2 = wpack[:, _OFF_WS2:_OFF_WS2 + HID]
        w_WuS = wpack[:, _OFF_WUS:_OFF_WUS + HID]
        w_WvS = wpack[:, _OFF_WVS:_OFF_WVS + HID]
        w_Wm2 = wpack[:, _OFF_WM2:_OFF_WM2 + HID]
        w_Wa1 = wpack[:, _OFF_WA1:_OFF_WA1 + HID]
        w_Wa2 = wpack[:, _OFF_WA2:_OFF_WA2 + 4]
        w_Ws1r = [wpack[0:1, _OFF_WS1 + c * HID:_OFF_WS1 + (c + 1) * HID]
                  for c in range(4)]
        w_Gur = [wpack[0:1, _OFF_GU + c * HID:_OFF_GU + (c + 1) * HID]
                 for c in range(3)]
        w_Gvr = [wpack[0:1, _OFF_GV + c * HID:_OFF_GV + (c + 1) * HID]
                 for c in range(3)]

        # per-partition bias columns [HID, 7]
        bcol = per.tile([HID, 7], dt.float32, tag="bcol")
        nc.sync.dma_start(out=bcol, in_=d_bias[:].rearrange("b h -> h b"))
        bs1 = bcol[:, 0:1]
        bs2 = bcol[:, 1:2]
        bm1 = bcol[:, 2:3]
        bm2 = bcol[:, 3:4]
        ba1 = bcol[:, 4:5]
        ba2mu = bcol[0:2, 5:6]
        ba2ls = bcol[0:2, 6:7]

        u_sb = per.tile([HID, S, NOBJ], edt, tag="u_sb")
        v_ext = per.tile([HID, S, 2 * NOBJ - 1], edt, tag="v_ext")
        agg = per.tile([HID, N], dt.float32, tag="agg")

        # BALL_REPEAT>1 re-runs the whole per-inference computation
        # (idempotent) so pipelined-call slope timing isolates device time.
        nrep = int(os.environ.get("BALL_REPEAT", "1"))
        ablate = os.environ.get("BALL_ABLATE", "none")
        for _rep in range(nrep):
          # ---- phase A: inputs -> node features u, v ----
          with tc.tile_pool(name="phA", bufs=1) as phA, \
               tc.tile_pool(name="psA", bufs=4, space="PSUM") as psA:

            state_nat = phA.tile([S, 63], dt.float32, tag="state_nat")
            nc.sync.dma_start(out=state_nat, in_=d_state[:])
            tar_nat = phA.tile([S, NOBJ * 2], dt.float32, tag="tar_nat")
            nc.sync.dma_start(out=tar_nat, in_=d_tar[:])

            # tanh(tar) in natural layout (cheap: 42 elems/partition)
            ttar_nat = phA.tile([S, NOBJ * 2], dt.float32, tag="ttar_nat")
            nc.scalar.activation(out=ttar_nat, in_=tar_nat, func=AF.Tanh)

            # one-hot of category in natural layout (exact in bf16)
            oh_nat = phA.tile([S, 3, NOBJ], edt, tag="oh_nat")
            cats_nat = state_nat[:].rearrange("s (i k) -> s k i", k=3)[:, 2, :]
            for c in range(3):
                nc.vector.tensor_scalar(
                    out=oh_nat[:, c, :], in0=cats_nat, scalar1=float(c),
                    scalar2=None, op0=OP.is_equal)

            # Stage spatial channels into a channel-blocked [s, k, i] tile
            # (two cheap in-partition DVE copies, casting to edt) so the
            # partition-collapse DMAs move contiguous 21-element runs
            # (128 descriptors each) instead of 4-byte scattered elements.
            st3 = state_nat[:].rearrange("s (i k) -> s k i", k=3)
            tt2 = ttar_nat[:].rearrange("s (i c) -> s c i", c=2)
            comb = phA.tile([S, 4, NOBJ], edt, tag="comb")
            nc.vector.tensor_copy(comb[:, 0:2, :], st3[:, 0:2, :])
            nc.vector.tensor_copy(comb[:, 2:4, :], tt2)
            spat_r = []
            for c in range(4):
                r = phA.tile([1, S, NOBJ], edt, tag=f"spat{c}")
                nc.sync.dma_start(out=r, in_=comb[:, c, :])
                spat_r.append(r)
            oh_r = []
            for c in range(3):
                r = phA.tile([1, S, NOBJ], edt, tag=f"oh{c}")
                nc.gpsimd.dma_start(out=r, in_=oh_nat[:, c, :])
                oh_r.append(r)

            h1 = phA.tile([HID, N], edt, tag="h1")
            feat = phA.tile([HID, N], edt, tag="feat")

            for s0, ns in chunks():
                cs = slice(s0 * NOBJ, (s0 + ns) * NOBJ)
                nn = ns * NOBJ
                p1 = psA.tile([HID, CH * NOBJ], dt.float32, tag="psA")
                for c in range(4):
                    nc.tensor.matmul(
                        p1[:, :nn], w_Ws1r[c],
                        spat_r[c][:].rearrange("o s i -> o (s i)")[:, cs],
                        start=(c == 0), stop=(c == 3))
                nc.scalar.activation(out=h1[:, cs], in_=p1[:, :nn],
                                     func=AF.Tanh, bias=bs1)
                p2 = psA.tile([HID, CH * NOBJ], dt.float32, tag="psA")
                nc.tensor.matmul(p2[:, :nn], w_Ws2, h1[:, cs],
                                 start=True, stop=True)
                nc.scalar.activation(out=feat[:, cs], in_=p2[:, :nn],
                                     func=AF.Tanh, bias=bs2)
                pu = psA.tile([HID, CH * NOBJ], dt.float32, tag="psA")
                nc.tensor.matmul(pu[:, :nn], w_WuS, feat[:, cs],
                                 start=True, stop=False)
                for c in range(3):
                    nc.tensor.matmul(
                        pu[:, :nn], w_Gur[c],
                        oh_r[c][:].rearrange("o s i -> o (s i)")[:, cs],
                        start=False, stop=(c == 2))
                nc.vector.tensor_copy(
                    u_sb[:].rearrange("c s i -> c (s i)")[:, cs], pu[:, :nn])
                pv = psA.tile([HID, CH * NOBJ], dt.float32, tag="psA")
                nc.tensor.matmul(pv[:, :nn], w_WvS, feat[:, cs],
                                 start=True, stop=False)
                for c in range(3):
                    nc.tensor.matmul(
                        pv[:, :nn], w_Gvr[c],
                        oh_r[c][:].rearrange("o s i -> o (s i)")[:, cs],
                        start=False, stop=(c == 2))
                nc.scalar.activation(
                    out=v_ext[:, s0:s0 + ns, 0:NOBJ],
                    in_=pv[:, :nn].rearrange("c (s i) -> c s i", i=NOBJ),
                    func=AF.Copy)
            # duplicate v columns so every cyclic shift is one strided AP
            nc.vector.tensor_copy(v_ext[:, :, NOBJ:], v_ext[:, :, 0:NOBJ - 1])

          # ---- phase B: all 420 edges/sample via 20 cyclic shifts ----
          LOOKAHEAD = 2
          with tc.tile_pool(name="psB", bufs=int(os.environ.get("BALL_PSB", "6")), space="PSUM") as psB:
            t_of = {}

            def produce(d):
                # half-granularity add/tanh so PE can start before the whole
                # column block is ready (latency hiding)
                if ablate == "noaddtanh":
                    t_of[d] = u_sb[:].rearrange("c s i -> c (s i)")
                    return
                h = edge.tile([HID, N], edt, tag="h")
                h3 = h[:].rearrange("c (s i) -> c s i", i=NOBJ)
                t = edge.tile([HID, N], edt, tag="t")
                half = S // 2
                for hs in range(2):
                    sl = slice(hs * half, (hs + 1) * half)
                    nc.vector.tensor_tensor(
                        out=h3[:, sl, :], in0=u_sb[:, sl, :],
                        in1=v_ext[:, sl, d:d + NOBJ], op=OP.add)
                    fl = slice(hs * half * NOBJ, (hs + 1) * half * NOBJ)
                    nc.scalar.activation(out=t[:, fl], in_=h[:, fl],
                                         func=AF.Tanh, bias=bm1)
                t_of[d] = t

            def consume(d):
                if ablate == "nomm":
                    return
                t = t_of.pop(d)
                for s0, ns in chunks():
                    cs = slice(s0 * NOBJ, (s0 + ns) * NOBJ)
                    nn = ns * NOBJ
                    pm = psB.tile([HID, CH * NOBJ], dt.float32, tag="msg")
                    nc.tensor.matmul(pm[:, :nn], w_Wm2, t[:, cs],
                                     start=True, stop=True)
                    if d == 1:
                        nc.vector.tensor_copy(agg[:, cs], pm[:, :nn])
                    elif ablate != "nomax":
                        nc.vector.tensor_tensor(
                            out=agg[:, cs], in0=agg[:, cs], in1=pm[:, :nn],
                            op=OP.max)

            for d in range(1, 1 + LOOKAHEAD):
                produce(d)
            for d in range(1, NOBJ):
                if d + LOOKAHEAD < NOBJ:
                    produce(d + LOOKAHEAD)
                consume(d)
            del t_of

            # ---- phase C: actor head ----
            # chunk-wise x so the head overlaps phase B's tail (each chunk
            # only needs the final max of its own columns)
            x = edge.tile([HID, N], edt, tag="h")
            a1 = edge.tile([HID, N], edt, tag="t")
            for s0, ns in chunks():
                cs = slice(s0 * NOBJ, (s0 + ns) * NOBJ)
                nn = ns * NOBJ
                nc.scalar.activation(out=x[:, cs], in_=agg[:, cs],
                                     func=AF.Tanh, bias=bm2)
                pa = psB.tile([HID, CH * NOBJ], dt.float32, tag="msg")
                nc.tensor.matmul(pa[:, :nn], w_Wa1, x[:, cs],
                                 start=True, stop=True)
                nc.scalar.activation(out=a1[:, cs], in_=pa[:, :nn],
                                     func=AF.Tanh, bias=ba1)
            ymu = per.tile([2, N], dt.float32, tag="ymu")
            yls = per.tile([2, N], dt.float32, tag="yls")
            for s0, ns in chunks():
                cs = slice(s0 * NOBJ, (s0 + ns) * NOBJ)
                nn = ns * NOBJ
                pmu = psB.tile([2, CH * NOBJ], dt.float32, tag="msg")
                nc.tensor.matmul(pmu[:, :nn], w_Wa2[:, 0:2], a1[:, cs],
                                 start=True, stop=True)
                nc.scalar.activation(out=ymu[:, cs], in_=pmu[:, :nn],
                                     func=AF.Tanh, bias=ba2mu)
                pls = psB.tile([2, CH * NOBJ], dt.float32, tag="msg")
                nc.tensor.matmul(pls[:, :nn], w_Wa2[:, 2:4], a1[:, cs],
                                 start=True, stop=True)
                nc.scalar.activation(out=yls[:, cs], in_=pls[:, :nn],
                                     func=AF.Tanh, bias=ba2ls)
            cneg = per.tile([2, 1], dt.float32, tag="cneg")
            nc.vector.memset(cneg, -1.5)
            mu_sb = per.tile([2, N], dt.float32, tag="mu_sb")
            nc.vector.tensor_scalar(out=mu_sb, in0=ymu, scalar1=0.3,
                                    scalar2=None, op0=OP.mult)
            std_sb = per.tile([2, N], dt.float32, tag="std_sb")
            nc.scalar.activation(out=std_sb, in_=yls, func=AF.Exp,
                                 bias=cneg, scale=3.5)

            # ---- output assembly: [S, 84] = [mu(i,c) | std(i,c)] ----
            # channel-blocked output [s, (mu_x|mu_y|std_x|std_y), i]:
            # every assembly DMA moves contiguous 21-element runs; the host
            # interleaves (i, c) afterwards.
            out_nat = per.tile([S, 4, NOBJ], dt.float32, tag="out_nat")
            mu3 = mu_sb[:].rearrange("c (s i) -> c s i", i=NOBJ)
            std3 = std_sb[:].rearrange("c (s i) -> c s i", i=NOBJ)
            for c in range(2):
                nc.sync.dma_start(out=out_nat[:, c, :], in_=mu3[c:c + 1, :, :])
                nc.sync.dma_start(out=out_nat[:, 2 + c, :],
                                  in_=std3[c:c + 1, :, :])
            # final store waits on 4 HWDGE lanes -> use SWDGE (flexible waits)
            nc.gpsimd.dma_start(out=d_out[:], in_=out_nat)

    nc.finalize()
    return nc


def _prep_params(inputs, edge_np):
    """Host-side pure parameter transforms (weights only, O(param size))."""
    f = lambda k: np.asarray(inputs[k], F32)
    Wm1 = f("Wm1")
    A, B = Wm1[:192], Wm1[192:]
    F3 = np.tanh(np.tanh(f("emb_table")) @ f("We") + f("be"))
    Gu = F3 @ (A[128:] - B[128:])
    Gv = F3 @ B[128:]

    wpack = np.zeros((HID, WCOLS), F32)
    wpack[:, _OFF_WS2:_OFF_WS2 + HID] = f("Ws2")
    wpack[:, _OFF_WUS:_OFF_WUS + HID] = A[:128] - B[:128]
    wpack[:, _OFF_WVS:_OFF_WVS + HID] = B[:128]
    wpack[:, _OFF_WM2:_OFF_WM2 + HID] = f("Wm2")
    wpack[:, _OFF_WA1:_OFF_WA1 + HID] = f("Wa1")
    wpack[:, _OFF_WA2:_OFF_WA2 + 4] = f("Wa2")
    Ws1 = f("Ws1")
    for c in range(4):
        wpack[0, _OFF_WS1 + c * HID:_OFF_WS1 + (c + 1) * HID] = Ws1[c]
    for c in range(3):
        wpack[0, _OFF_GU + c * HID:_OFF_GU + (c + 1) * HID] = Gu[c]
        wpack[0, _OFF_GV + c * HID:_OFF_GV + (c + 1) * HID] = Gv[c]

    ba2 = f("ba2")
    ba2mu = np.zeros(HID, F32)
    ba2mu[:2] = ba2[:2]
    ba2ls = np.zeros(HID, F32)
    ba2ls[:2] = ba2[2:]
    biases = np.stack([f("bs1"), f("bs2"), f("bm1"), f("bm2"), f("ba1"),
                       ba2mu, ba2ls])
    return dict(
        wpack=np.ascontiguousarray(wpack.astype(edge_np)),
        biases=np.ascontiguousarray(biases),
    )


def kernel(**inputs):
    from concourse.bass_utils import run_bass_kernel_spmd

    edge_dt_name = os.environ.get("BALL_EDGE_DT", "bfloat16")
    trace = os.environ.get("BALL_TRACE", "0") == "1"

    if edge_dt_name not in _cache:
        _cache[edge_dt_name] = _build_nc(edge_dt_name)
    nc = _cache[edge_dt_name]

    edge_np = {"bfloat16": BF16, "float32": F32}[edge_dt_name]
    params = _prep_params(inputs, edge_np)

    state = np.ascontiguousarray(np.asarray(inputs["state_inp"], F32))
    tar = np.asarray(inputs["tar_scores"], F32).reshape(BS, NOBJ * 2)

    in_maps = []
    for c in range(NCORES):
        m = dict(params)
        m["state"] = state[c * S:(c + 1) * S]
        m["tar"] = np.ascontiguousarray(tar[c * S:(c + 1) * S])
        in_maps.append(m)

    res = run_bass_kernel_spmd(nc, in_maps, core_ids=list(range(NCORES)),
                               trace=trace)
    kernel.last_results = res

    outs = [res.results[c]["out"] for c in range(NCORES)]
    full = np.concatenate(outs, axis=0).reshape(BS, 4, NOBJ)
    mu = np.ascontiguousarray(
        full[:, 0:2, :].transpose(0, 2, 1).reshape(BS, 2 * NOBJ))
    std = np.ascontiguousarray(
        full[:, 2:4, :].transpose(0, 2, 1).reshape(BS, 2 * NOBJ))
    return mu, std



# revision 16
# speedup vs baseline: 2.4977x; 2.4977x over previous
"""Trainium2 Bass kernel for the BallActor GNN (EdgeConv over fully-connected
per-sample graphs, batch 1024 x 21 objects).

Key algorithmic facts exploited:
  * knn_actor K=20 over NOBJ=21 with self masked => the "kNN graph" is simply
    ALL ordered pairs (i, j != i); top_k is unnecessary and max-aggregation is
    order independent.
  * EdgeConv first layer is linear in [x_i, x_j - x_i]:
        h(i,j) = x_i @ (A - B) + x_j @ B + bm1   (Wm1 = [[A],[B]])
    so per-node terms u_i = x_i@(A-B), v_j = x_j@B are precomputed and each
    edge costs only an elementwise add + tanh + the second 128x128 matmul.
  * The class embedding path collapses to a 3-row table lookup, folded into
    u/v via one-hot rows (host precomputes F3 = tanh(tanh(emb)@We + be) and
    G = F3 @ W_cls); applied as one K=3 matmul against a 3-partition one-hot.
  * Edges are enumerated as 20 cyclic shifts d=1..20: j = (i+d) mod 21.  v is
    stored duplicated along the object axis with an even row stride (42) in
    TWO parity copies so every shift window starts 4B-aligned => the DVE adds
    run in the 2x bf16 perf mode.
  * The actor-head output layer is transposed on the PE (node blocks of 84 =
    4 samples x 21 on the partition axis) so the tail activations process
    FD=32 instead of FD=2688 per instruction.

Sharding: pure data parallel over the batch: 1024 samples -> 8 cores x 128.
Params are replicated; outputs are concatenated on host.
"""

import os
import numpy as np
import ml_dtypes

BS = 1024
NOBJ = 21
HID = 128
EMB = 64
NCORES = 8
S = BS // NCORES          # samples per core
N = S * NOBJ              # nodes per core (2688)
CH = 24                   # samples per phase-A matmul chunk (24*21 = 504)
F32 = np.float32
BF16 = ml_dtypes.bfloat16

# phase-B consume chunking: bank-aligned psum tiles of 1024 fp32 (2 banks)
BCH = (1024, 1024, 640)
BOFF = (0, 1024, 2048)

# weight-pack column layout (single [128, WCOLS] tensor, one DMA)
_OFF_WS2 = 0
_OFF_WUS = 128
_OFF_WVS = 256
_OFF_WM2 = 384
_OFF_WA1 = 512
_OFF_WA2 = 640            # 4 cols
_OFF_WS1 = 644            # [4, 128] on partitions 0..3
_OFF_GU = 644 + 128       # [3, 128] on partitions 0..2
_OFF_GV = _OFF_GU + 128
WCOLS = _OFF_GV + 128

_cache = {}


def _build_nc(edge_dt_name: str):
    import concourse.bass as bass  # noqa: F401
    import concourse.bacc as bacc
    import concourse.tile as tile
    from concourse import mybir

    dt = mybir.dt
    edt = getattr(dt, edge_dt_name)
    AF = mybir.ActivationFunctionType
    OP = mybir.AluOpType

    nc = bacc.Bacc("TRN2")

    # ---------------- DRAM I/O ----------------
    d_state = nc.dram_tensor("state", [S, 63], dt.float32, kind="ExternalInput")
    d_tar = nc.dram_tensor("tar", [S, NOBJ * 2], dt.float32, kind="ExternalInput")
    d_wpack = nc.dram_tensor("wpack", [HID, WCOLS], edt, kind="ExternalInput")
    # bias rows: bs1, bs2, bm1, bm2, ba1, ba2[0], ba2[1], ba2[2], ba2[3]
    d_bias = nc.dram_tensor("biases", [9, HID], dt.float32, kind="ExternalInput")
    # output rows s, cols (h, i, c): h=0 mu, h=1 std; host reshapes
    d_out = nc.dram_tensor("out", [S, 4 * NOBJ], dt.float32, kind="ExternalOutput")

    nchunk = (S + CH - 1) // CH  # 6 chunks: 5 full + one 8-sample remainder

    def chunks():
        for k in range(nchunk):
            s0 = k * CH
            ns = min(CH, S - s0)
            yield s0, ns

    n_gs = int(os.environ.get("BALL_GS", "4"))      # adds on gpsimd
    gs_set = set()
    if n_gs > 0:
        step = 20 / n_gs
        gs_set = {1 + int(round(k * step)) for k in range(n_gs)}
        gs_set = {d for d in gs_set if 1 <= d <= 20}
    # shifts whose msg psum is evacuated by ACT (bf16) and max'd at 2x
    n_aa = int(os.environ.get("BALL_ACTA", "0"))
    aa_set = set(range(20 - n_aa + 1, 20 + 1)) if n_aa > 0 else set()

    with tile.TileContext(nc) as tc, \
         tc.tile_pool(name="per", bufs=1) as per, \
         tc.tile_pool(name="edge", bufs=4) as edge:

        # ---- persistent tiles ----
        wpack = per.tile([HID, WCOLS], edt, tag="wpack")
        nc.sync.dma_start(out=wpack, in_=d_wpack[:])
        w_Ws2 = wpack[:, _OFF_WS2:_OFF_WS2 + HID]
        w_WuS = wpack[:, _OFF_WUS:_OFF_WUS + HID]
        w_WvS = wpack[:, _OFF_WVS:_OFF_WVS + HID]
        w_Wm2 = wpack[:, _OFF_WM2:_OFF_WM2 + HID]
        w_Wa1 = wpack[:, _OFF_WA1:_OFF_WA1 + HID]
        w_Wa2 = wpack[:, _OFF_WA2:_OFF_WA2 + 4]
        w_Ws1 = wpack[0:4, _OFF_WS1:_OFF_WS1 + HID]
        w_Gu = wpack[0:3, _OFF_GU:_OFF_GU + HID]
        w_Gv = wpack[0:3, _OFF_GV:_OFF_GV + HID]

        # per-partition bias columns [HID, 9]
        bcol = per.tile([HID, 9], dt.float32, tag="bcol")
        nc.sync.dma_start(out=bcol, in_=d_bias[:].rearrange("b h -> h b"))
        bs1 = bcol[:, 0:1]
        bs2 = bcol[:, 1:2]
        bm1 = bcol[:, 2:3]
        bm2 = bcol[:, 3:4]
        ba1 = bcol[:, 4:5]
        ba2c = [bcol[:, 5 + c:6 + c] for c in range(4)]

        u_sb = per.tile([HID, S, NOBJ], edt, tag="u_sb")
        # v duplicated along objects, even row stride, two parity copies so
        # every shift window is 4B-aligned (keeps DVE adds in 2x mode)
        v_e = per.tile([HID, S, 42], edt, tag="v_e")
        v_o = per.tile([HID, S, 42], edt, tag="v_o")
        agg = per.tile([HID, N], dt.float32, tag="agg")

        # BALL_REPEAT>1 re-runs the whole per-inference computation
        # (idempotent) so pipelined-call slope timing isolates device time.
        nrep = int(os.environ.get("BALL_REPEAT", "1"))
        ablate = os.environ.get("BALL_ABLATE", "none")
        for _rep in range(nrep):
          # ---- phase A: inputs -> node features u, v ----
          with tc.tile_pool(name="phA", bufs=1) as phA, \
               tc.tile_pool(name="psA", bufs=4, space="PSUM") as psA:

            state_nat = phA.tile([S, 63], dt.float32, tag="state_nat")
            nc.sync.dma_start(out=state_nat, in_=d_state[:])
            tar_nat = phA.tile([S, NOBJ * 2], dt.float32, tag="tar_nat")
            nc.sync.dma_start(out=tar_nat, in_=d_tar[:])

            # tanh(tar) in natural layout (cheap: 42 elems/partition)
            ttar_nat = phA.tile([S, NOBJ * 2], dt.float32, tag="ttar_nat")
            nc.scalar.activation(out=ttar_nat, in_=tar_nat, func=AF.Tanh)

            # one-hot of category in natural layout (exact in bf16)
            oh_nat = phA.tile([S, 3, NOBJ], edt, tag="oh_nat")
            cats_nat = state_nat[:].rearrange("s (i k) -> s k i", k=3)[:, 2, :]
            for c in range(3):
                nc.vector.tensor_scalar(
                    out=oh_nat[:, c, :], in0=cats_nat, scalar1=float(c),
                    scalar2=None, op0=OP.is_equal)

            # Stage spatial channels into a channel-blocked [s, k, i] tile
            # (two cheap in-partition DVE copies, casting to edt) so the
            # partition-collapse DMAs move contiguous 21-element runs.
            st3 = state_nat[:].rearrange("s (i k) -> s k i", k=3)
            tt2 = ttar_nat[:].rearrange("s (i c) -> s c i", c=2)
            comb = phA.tile([S, 4, NOBJ], edt, tag="comb")
            nc.vector.tensor_copy(comb[:, 0:2, :], st3[:, 0:2, :])
            nc.vector.tensor_copy(comb[:, 2:4, :], tt2)
            # channel-major staging: [4, S*NOBJ] and [3, S*NOBJ]
            # (spread across DMA queues so descriptor-gen latencies overlap)
            spat4 = phA.tile([4, S, NOBJ], edt, tag="spat4")
            for c, q in zip(range(4), (nc.sync, nc.scalar, nc.sync, nc.scalar)):
                q.dma_start(out=spat4[c:c + 1], in_=comb[:, c, :])
            oh3 = phA.tile([3, S, NOBJ], edt, tag="oh3")
            for c, q in zip(range(3), (nc.gpsimd, nc.sync, nc.scalar)):
                q.dma_start(out=oh3[c:c + 1], in_=oh_nat[:, c, :])
            spat4f = spat4[:].rearrange("c s i -> c (s i)")
            oh3f = oh3[:].rearrange("c s i -> c (s i)")

            h1 = phA.tile([HID, N], edt, tag="h1")
            feat = phA.tile([HID, N], edt, tag="feat")

            for s0, ns in chunks():
                cs = slice(s0 * NOBJ, (s0 + ns) * NOBJ)
                nn = ns * NOBJ
                p1 = psA.tile([HID, CH * NOBJ], dt.float32, tag="psA")
                nc.tensor.matmul(p1[:, :nn], w_Ws1, spat4f[:, cs],
                                 start=True, stop=True)
                nc.scalar.activation(out=h1[:, cs], in_=p1[:, :nn],
                                     func=AF.Tanh, bias=bs1)
                p2 = psA.tile([HID, CH * NOBJ], dt.float32, tag="psA")
                nc.tensor.matmul(p2[:, :nn], w_Ws2, h1[:, cs],
                                 start=True, stop=True)
                nc.scalar.activation(out=feat[:, cs], in_=p2[:, :nn],
                                     func=AF.Tanh, bias=bs2)
                pu = psA.tile([HID, CH * NOBJ], dt.float32, tag="psA")
                nc.tensor.matmul(pu[:, :nn], w_WuS, feat[:, cs],
                                 start=True, stop=False)
                nc.tensor.matmul(pu[:, :nn], w_Gu, oh3f[:, cs],
                                 start=False, stop=True)
                nc.scalar.activation(
                    out=u_sb[:].rearrange("c s i -> c (s i)")[:, cs],
                    in_=pu[:, :nn], func=AF.Copy)
                pv = psA.tile([HID, CH * NOBJ], dt.float32, tag="psA")
                nc.tensor.matmul(pv[:, :nn], w_WvS, feat[:, cs],
                                 start=True, stop=False)
                nc.tensor.matmul(pv[:, :nn], w_Gv, oh3f[:, cs],
                                 start=False, stop=True)
                nc.vector.tensor_copy(
                    v_e[:, s0:s0 + ns, 0:NOBJ],
                    pv[:, :nn].rearrange("c (s i) -> c s i", i=NOBJ))
            # duplicate v columns so every cyclic shift is one strided AP,
            # then build the odd-parity copy (shifted left by one element)
            nc.vector.tensor_copy(v_e[:, :, NOBJ:2 * NOBJ - 1],
                                  v_e[:, :, 0:NOBJ - 1])
            nc.vector.tensor_copy(v_o[:, :, 0:2 * NOBJ - 2],
                                  v_e[:, :, 1:2 * NOBJ - 1])

          # ---- phase B: all 420 edges/sample via 20 cyclic shifts ----
          LOOKAHEAD = 2
          with tc.tile_pool(name="psB", bufs=3, space="PSUM") as psB, \
               tc.tile_pool(name="psT", bufs=1, space="PSUM") as psT:
            t_of = {}

            def produce(d):
                if ablate == "noaddtanh":
                    t_of[d] = u_sb[:].rearrange("c s i -> c (s i)")
                    return
                par = d & 1
                vsrc = v_o if par else v_e
                c0 = d - par
                h = edge.tile([HID, S, NOBJ], edt, tag="h")
                eng = nc.gpsimd if d in gs_set else nc.vector
                eng.tensor_tensor(
                    out=h, in0=u_sb, in1=vsrc[:, :, c0:c0 + NOBJ], op=OP.add)
                t = edge.tile([HID, N], edt, tag="t")
                nc.scalar.activation(
                    out=t, in_=h[:].rearrange("c s i -> c (s i)"),
                    func=AF.Tanh, bias=bm1)
                t_of[d] = t

            aggb = None
            if aa_set:
                aggb = per.tile([HID, N], edt, tag="aggb")

            def consume(d):
                if ablate == "nomm":
                    return
                t = t_of.pop(d)
                assisted = d in aa_set
                mb = None
                if assisted:
                    mb = edge.tile([HID, N], edt, tag="mb", name=f"mb{d}")
                for c0, cw in zip(BOFF, BCH):
                    cs = slice(c0, c0 + cw)
                    pm = psB.tile([HID, 1024], dt.float32, tag="msg")
                    nc.tensor.matmul(pm[:, 0:512], w_Wm2, t[:, c0:c0 + 512],
                                     start=True, stop=True)
                    if cw > 512:
                        nc.tensor.matmul(pm[:, 512:cw], w_Wm2,
                                         t[:, c0 + 512:c0 + cw],
                                         start=True, stop=True)
                    if d == 1:
                        nc.scalar.activation(out=agg[:, cs], in_=pm[:, :cw],
                                             func=AF.Copy)
                    elif ablate == "nomax":
                        pass
                    elif assisted:
                        nc.scalar.activation(out=mb[:, cs], in_=pm[:, :cw],
                                             func=AF.Copy)
                    else:
                        nc.vector.tensor_tensor(
                            out=agg[:, cs], in0=agg[:, cs], in1=pm[:, :cw],
                            op=OP.max)
                if assisted:
                    aeng = (nc.gpsimd
                            if os.environ.get("BALL_AAENG", "vector") == "gpsimd"
                            else nc.vector)
                    if d == min(aa_set):
                        aeng.tensor_copy(aggb, mb)
                    else:
                        aeng.tensor_tensor(out=aggb, in0=aggb, in1=mb,
                                           op=OP.max)

            for d in range(1, 1 + LOOKAHEAD):
                produce(d)
            for d in range(1, NOBJ):
                if d + LOOKAHEAD < NOBJ:
                    produce(d + LOOKAHEAD)
                consume(d)
            del t_of

            # ---- phase C: actor head ----
            if aa_set and ablate == "none":
                nc.vector.tensor_tensor(out=agg, in0=agg, in1=aggb, op=OP.max)
            x = edge.tile([HID, N], edt, tag="h")
            a1 = edge.tile([HID, N], edt, tag="t")
            for c0, cw in zip(BOFF, BCH):
                cs = slice(c0, c0 + cw)
                nc.scalar.activation(out=x[:, cs], in_=agg[:, cs],
                                     func=AF.Tanh, bias=bm2)
                pa = psB.tile([HID, 1024], dt.float32, tag="msg")
                nc.tensor.matmul(pa[:, 0:512], w_Wa1, x[:, c0:c0 + 512],
                                 start=True, stop=True)
                if cw > 512:
                    nc.tensor.matmul(pa[:, 512:cw], w_Wa1,
                                     x[:, c0 + 512:c0 + cw],
                                     start=True, stop=True)
                nc.scalar.activation(out=a1[:, cs], in_=pa[:, :cw],
                                     func=AF.Tanh, bias=ba1)

            # transposed output layer: 32 node-blocks of 84 (= 4 samples),
            # partitions become node instances, 4 head outputs per block
            NBLK = 32
            BW = 84
            pT = psT.tile([HID, 4 * NBLK], dt.float32, tag="pT")
            for b in range(NBLK):
                nc.tensor.matmul(pT[0:BW, 4 * b:4 * b + 4],
                                 a1[:, BW * b:BW * (b + 1)], w_Wa2,
                                 start=True, stop=True)
            pT4 = pT[0:BW].rearrange("p (b c) -> p b c", c=4)
            # combined output staging [84p, 32b, 2h, 2c]
            osb = per.tile([HID, NBLK, 2, 2], dt.float32, tag="osb")
            tmu = per.tile([HID, NBLK, 2], edt, tag="tmu")
            for c in range(2):
                nc.scalar.activation(out=tmu[0:BW, :, c], in_=pT4[:, :, c],
                                     func=AF.Tanh, bias=ba2c[c][0:BW])
            nc.vector.tensor_scalar(
                out=osb[0:BW, :, 0, :], in0=tmu[0:BW], scalar1=0.3,
                scalar2=None, op0=OP.mult)
            dview = d_out[:].rearrange("(b q) (h i c) -> q h i b c",
                                       q=4, h=2, c=2)
            for q in range(4):
                nc.sync.dma_start(out=dview[q, 0],
                                  in_=osb[q * NOBJ:(q + 1) * NOBJ, :, 0, :])
            tls = per.tile([HID, NBLK, 2], edt, tag="tls")
            for c in range(2):
                nc.scalar.activation(out=tls[0:BW, :, c], in_=pT4[:, :, 2 + c],
                                     func=AF.Tanh, bias=ba2c[2 + c][0:BW])
            cneg = per.tile([HID, 1], dt.float32, tag="cneg")
            nc.vector.memset(cneg, -1.5)
            nc.scalar.activation(out=osb[0:BW, :, 1, :], in_=tls[0:BW],
                                 func=AF.Exp, bias=cneg[0:BW], scale=3.5)

            # ---- output DMA (std half; mu half already issued above) ----
            for q in range(4):
                nc.sync.dma_start(out=dview[q, 1],
                                  in_=osb[q * NOBJ:(q + 1) * NOBJ, :, 1, :])

    nc.finalize()
    return nc


def _prep_params(inputs, edge_np):
    """Host-side pure parameter transforms (weights only, O(param size))."""
    f = lambda k: np.asarray(inputs[k], F32)
    Wm1 = f("Wm1")
    A, B = Wm1[:192], Wm1[192:]
    F3 = np.tanh(np.tanh(f("emb_table")) @ f("We") + f("be"))
    Gu = F3 @ (A[128:] - B[128:])
    Gv = F3 @ B[128:]

    wpack = np.zeros((HID, WCOLS), F32)
    wpack[:, _OFF_WS2:_OFF_WS2 + HID] = f("Ws2")
    wpack[:, _OFF_WUS:_OFF_WUS + HID] = A[:128] - B[:128]
    wpack[:, _OFF_WVS:_OFF_WVS + HID] = B[:128]
    wpack[:, _OFF_WM2:_OFF_WM2 + HID] = f("Wm2")
    wpack[:, _OFF_WA1:_OFF_WA1 + HID] = f("Wa1")
    wpack[:, _OFF_WA2:_OFF_WA2 + 4] = f("Wa2")
    wpack[0:4, _OFF_WS1:_OFF_WS1 + HID] = f("Ws1")
    wpack[0:3, _OFF_GU:_OFF_GU + HID] = Gu
    wpack[0:3, _OFF_GV:_OFF_GV + HID] = Gv

    ba2 = f("ba2")
    biases = np.stack([f("bs1"), f("bs2"), f("bm1"), f("bm2"), f("ba1")]
                      + [np.full(HID, ba2[c], F32) for c in range(4)])
    return dict(
        wpack=np.ascontiguousarray(wpack.astype(edge_np)),
        biases=np.ascontiguousarray(biases),
    )


def kernel(**inputs):
    from concourse.bass_utils import run_bass_kernel_spmd

    edge_dt_name = os.environ.get("BALL_EDGE_DT", "bfloat16")
    trace = os.environ.get("BALL_TRACE", "0") == "1"

    if edge_dt_name not in _cache:
        _cache[edge_dt_name] = _build_nc(edge_dt_name)
    nc = _cache[edge_dt_name]

    edge_np = {"bfloat16": BF16, "float32": F32}[edge_dt_name]
    params = _prep_params(inputs, edge_np)

    state = np.ascontiguousarray(np.asarray(inputs["state_inp"], F32))
    tar = np.asarray(inputs["tar_scores"], F32).reshape(BS, NOBJ * 2)

    in_maps = []
    for c in range(NCORES):
        m = dict(params)
        m["state"] = state[c * S:(c + 1) * S]
        m["tar"] = np.ascontiguousarray(tar[c * S:(c + 1) * S])
        in_maps.append(m)

    res = run_bass_kernel_spmd(nc, in_maps, core_ids=list(range(NCORES)),
                               trace=trace)
    kernel.last_results = res

    outs = [res.results[c]["out"] for c in range(NCORES)]
    full = np.concatenate(outs, axis=0).reshape(BS, 2, NOBJ, 2)
    mu = np.ascontiguousarray(full[:, 0].reshape(BS, 2 * NOBJ))
    std = np.ascontiguousarray(full[:, 1].reshape(BS, 2 * NOBJ))
    return mu, std
